# revision 18
# baseline (speedup 1.0000x reference)
"""Causal self-attention (softmax over the QUERY axis) for Trainium2, 8 cores.

Reference semantics (note the quirk -- softmax over dim=1, the query axis):
    q = x @ Wq.T ; k = x @ Wk.T ; v = x @ Wv.T          (per batch)
    s[q_, k_] = <q[q_], k[k_]>,  masked -inf where k_ > q_
    attn = softmax(s / sqrt(D), axis=q_)                 (normalize per key column)
    out[q_, :] = sum_k attn[q_, k_] * v[k_, :]

Because the softmax normalizes each key COLUMN over queries, the whole thing
factors as  out = W @ (v / Z)  with
    W[k_, q_] = exp(s^T * scale) * causal_mask,   Z[k_] = sum_q W[k_, q_].

Algebraic folding: s[q_, k_] = x[q_] . A . x[k_]  with A = Wq^T @ Wk, so with
y = x_k @ A^T the scores come straight from x (no q/k projections needed):
    s^T[k_, q_] = sum_d y[k_, d] * x[q_, d].
A is computed once on the host.

Sharding: 8 cores = 4 batches x 2 key-shards.  Key columns are interleaved by
parity (core h in {0,1} owns original key positions 2*m + h) so the causal
work balances AND every core runs the identical program (pure SPMD); only the
input data differs per core.  Each core computes a partial output (sum over
its own keys); the host adds the two partials per batch.

Device layout (per core, b = batch, h = parity):
    xT   [D, N]  bf16  x[b].T
    a2   [D, D]  bf16  A^T = Wk^T @ Wq   (layout [e, d])
    wvT  [D, D]  bf16  Wv.T              (layout [e, o])
    maskbias [128, 256] f32  0 where valid, -1e9 where masked (depends on h)
    out  [N, D]  f32   partial output

All matmul inputs are bf16 (PE full rate), accumulation fp32 in PSUM.
"""
import numpy as np
import ml_dtypes
from contextlib import ExitStack

import concourse.bass as bass
import concourse.tile as tile
import concourse.bacc as bacc
import concourse.mybir as mybir
from concourse.bass_utils import run_bass_kernel_spmd

B, N, D = 4, 2048, 1024
NT = N // 128          # 16 query tiles
ET = D // 128          # 8 contraction tiles
G = 8                  # key groups per core (128 interleaved keys each)
SCALE = 1.0 / np.sqrt(D).astype(np.float32)
NEGBIG = -1.0e9

BF = mybir.dt.bfloat16
F32 = mybir.dt.float32

# packed offsets of each group's score row-block inside the wT buffer
WOFF = []
_o = 0
for _g in range(G):
    WOFF.append(_o)
    _o += N - 256 * _g
WTOT = _o  # 9216


def _score_chunks(g):
    """(q0, width) chunks covering the valid span [256g, N) of group g.
    The first chunk always contains the 256 masked columns; widths 512/256."""
    width = N - 256 * g
    q0 = 256 * g
    chunks = []
    if (width // 256) % 2 == 1:
        chunks.append((q0, 256))
        q0 += 256
    while q0 < N:
        chunks.append((q0, 512))
        q0 += 512
    return chunks


def _emit_body(nc, tc, ctx, pools, aps, stages="full", mm_tiles=None):
    (xpool, wpool, ypool, vpool, vppool, zpool, stpool, ps, avps) = pools
    (xT_d, xkT_d, a2_d, wvT_d, mb_sb, wt_sb, out_d) = aps

    # ---- loads: what y-proj needs first (xk + a2), then wvT, then xT ----
    xk = []
    for t in range(ET):
        xktile = xpool.tile([128, D], BF, tag="xk")
        nc.sync.dma_start(xktile[:], xkT_d[t * 128:(t + 1) * 128, :])
        xk.append(xktile)
    a2t = []
    for t in range(ET):
        w = wpool.tile([128, D], BF, tag="w")
        nc.sync.dma_start(w[:], a2_d[t * 128:(t + 1) * 128, :])
        a2t.append(w)
    wvt = []
    for t in range(ET):
        w = wpool.tile([128, D], BF, tag="w")
        nc.sync.dma_start(w[:], wvT_d[t * 128:(t + 1) * 128, :])
        wvt.append(w)
    xt = []
    for t in range(ET):
        xtile = xpool.tile([128, N], BF, tag="xt")
        nc.sync.dma_start(xtile[:], xT_d[t * 128:(t + 1) * 128, :])
        xt.append(xtile)

    # ---- y projection: yT[d_tile][:, m] = sum_e a2[e, d] * xkT[e, m] ----
    yt = []
    for dt_ in range(ET):
        ytile = (ypool.tile([128, D], BF, tag="yt") if stages == "full"
                 else mm_tiles[0][dt_])            # owned m = 1024 cols
        for c in range(2):                            # m chunks of 512
            pt = ps.tile([128, 512], F32, tag="ps")
            for e in range(ET):
                nc.tensor.matmul(
                    pt[:],
                    a2t[e][:, dt_ * 128:(dt_ + 1) * 128],
                    xk[e][:, c * 512:(c + 1) * 512],
                    start=(e == 0), stop=(e == ET - 1),
                )
            if stages == "full":
                nc.vector.tensor_copy(ytile[:, c * 512:(c + 1) * 512], pt[:])
        yt.append(ytile)

    # ---- v projection: v[g][i, o] = sum_e xkT[e, 128g+i] * wvT[e, o] ----
    vt = []
    for g in range(G):
        vtile = vpool.tile([128, D], F32, tag="v")
        for c in range(2):                            # o chunks of 512
            pt = ps.tile([128, 512], F32, tag="ps")
            for e in range(ET):
                nc.tensor.matmul(
                    pt[:],
                    xk[e][:, g * 128:(g + 1) * 128],
                    wvt[e][:, c * 512:(c + 1) * 512],
                    start=(e == 0), stop=(e == ET - 1),
                )
            if stages == "full":
                nc.vector.tensor_copy(vtile[:, c * 512:(c + 1) * 512], pt[:])
        vt.append(vtile)

    # ---- per group: scores^T -> exp/mask/Z -> v' ; then AV for its q-tiles ----
    vp = [None] * G
    for g in range(G):
        chunks = _score_chunks(g)
        nch = len(chunks)
        if stages == "full":
            zp = zpool.tile([128, nch], F32, tag="zp")
        else:
            zp = None
        for ci, (q0, w) in enumerate(chunks):
            pt = ps.tile([128, 512], F32, tag="ps")
            for dt_ in range(ET):
                lhs = yt[dt_][:, g * 128:(g + 1) * 128]
                nc.tensor.matmul(
                    pt[:, :w],
                    lhs,
                    xt[dt_][:, q0:q0 + w],
                    start=(dt_ == 0), stop=(dt_ == ET - 1),
                )
            if stages == "full":
                if ci == 0:
                    # masked (diagonal) region = first 256 valid columns
                    nc.vector.tensor_add(pt[:, :256], pt[:, :256], mb_sb[:])
                nc.scalar.activation(
                    wt_sb[:, WOFF[g] + (q0 - 256 * g): WOFF[g] + (q0 - 256 * g) + w],
                    pt[:, :w],
                    mybir.ActivationFunctionType.Exp,
                    scale=float(SCALE),
                    accum_out=zp[:, ci:ci + 1],
                )
        if stages != "full":
            vptile = mm_tiles[1][g]
        else:
            vptile = vppool.tile([128, D], BF, tag="vp")
        if stages == "full":
            z = zpool.tile([128, 1], F32, tag="z")
            nc.vector.tensor_reduce(z[:], zp[:], axis=mybir.AxisListType.X,
                                    op=mybir.AluOpType.add)
            rz = zpool.tile([128, 1], F32, tag="rz")
            nc.vector.reciprocal(rz[:], z[:])
            nc.vector.tensor_scalar_mul(vptile[:], vt[g][:], rz[:])
        vp[g] = vptile

        # AV for q-tiles 2g and 2g+1 (they need groups 0..g only)
        for qt in (2 * g, 2 * g + 1):
            stage = stpool.tile([128, D], F32, tag="st")
            for oc in range(2):
                apt = avps.tile([128, 512], F32, tag="av")
                for gg in range(g + 1):
                    lhs = wt_sb[:, WOFF[gg] + 128 * qt - 256 * gg:
                                   WOFF[gg] + 128 * qt - 256 * gg + 128]
                    rhs = vp[gg][:, oc * 512:(oc + 1) * 512]
                    nc.tensor.matmul(apt[:], lhs, rhs,
                                     start=(gg == 0), stop=(gg == g))
                if stages == "full" or qt == NT - 1:
                    nc.vector.tensor_copy(stage[:, oc * 512:(oc + 1) * 512], apt[:])
            if stages == "full" or qt == NT - 1:
                nc.sync.dma_start(out_d[qt * 128:(qt + 1) * 128, :], stage[:])


def _emit_body_v2(nc, tc, ctx, pools, aps):
    """v2: weights (a2/wvT) are persistent (loaded outside the loop); AV for
    q-tile pair g-1 is emitted after scores/exp of group g (lag-1 software
    pipeline) so the exp/Z/vp chain of a group hides under the next group's
    score matmuls; AV stage copies go on the Activation engine."""
    (xpool, ypool, vpool, vppool, zpool, stpool, ps, avps) = pools
    (xT_d, xkT_d, a2t, wvt, mb_sb, wt_sb, out_d) = aps

    xk = []
    for t in range(ET):
        xktile = xpool.tile([128, D], BF, tag="xk", name="xk")
        nc.sync.dma_start(xktile[:], xkT_d[t * 128:(t + 1) * 128, :])
        xk.append(xktile)
    xt = []
    for t in range(ET):
        xtile = xpool.tile([128, N], BF, tag="xt", name="xt")
        nc.sync.dma_start(xtile[:], xT_d[t * 128:(t + 1) * 128, :])
        xt.append(xtile)

    # ---- y projection ----
    yt = []
    for dt_ in range(ET):
        ytile = ypool.tile([128, D], BF, tag="yt", name="yt")
        for c in range(2):
            pt = ps.tile([128, 512], F32, tag="ps", name="pt")
            for e in range(ET):
                nc.tensor.matmul(
                    pt[:],
                    a2t[e][:, dt_ * 128:(dt_ + 1) * 128],
                    xk[e][:, c * 512:(c + 1) * 512],
                    start=(e == 0), stop=(e == ET - 1),
                )
            nc.vector.tensor_copy(ytile[:, c * 512:(c + 1) * 512], pt[:])
        yt.append(ytile)

    # ---- v projection ----
    vt = []
    for g in range(G):
        vtile = vpool.tile([128, D], F32, tag="v", name="vt")
        for c in range(2):
            pt = ps.tile([128, 512], F32, tag="ps", name="pt")
            for e in range(ET):
                nc.tensor.matmul(
                    pt[:],
                    xk[e][:, g * 128:(g + 1) * 128],
                    wvt[e][:, c * 512:(c + 1) * 512],
                    start=(e == 0), stop=(e == ET - 1),
                )
            nc.vector.tensor_copy(vtile[:, c * 512:(c + 1) * 512], pt[:])
        vt.append(vtile)

    vp = [None] * G

    def emit_av(g):
        for qt in (2 * g, 2 * g + 1):
            stage = stpool.tile([128, D], F32, tag="st", name="st")
            for oc in range(2):
                apt = avps.tile([128, 512], F32, tag="av", name="apt")
                for gg in range(g + 1):
                    lhs = wt_sb[:, WOFF[gg] + 128 * qt - 256 * gg:
                                   WOFF[gg] + 128 * qt - 256 * gg + 128]
                    rhs = vp[gg][:, oc * 512:(oc + 1) * 512]
                    nc.tensor.matmul(apt[:], lhs, rhs,
                                     start=(gg == 0), stop=(gg == g))
                nc.scalar.activation(stage[:, oc * 512:(oc + 1) * 512],
                                     apt[:],
                                     mybir.ActivationFunctionType.Copy)
            nc.sync.dma_start(out_d[qt * 128:(qt + 1) * 128, :], stage[:])

    # ---- per group: scores -> exp/Z -> vp ; AV lags one group behind ----
    for g in range(G):
        chunks = _score_chunks(g)
        nch = len(chunks)
        zp = zpool.tile([128, nch], F32, tag="zp", name="zp")
        for ci, (q0, w) in enumerate(chunks):
            pt = ps.tile([128, 512], F32, tag="ps", name="pt")
            for dt_ in range(ET):
                nc.tensor.matmul(
                    pt[:, :w],
                    yt[dt_][:, g * 128:(g + 1) * 128],
                    xt[dt_][:, q0:q0 + w],
                    start=(dt_ == 0), stop=(dt_ == ET - 1),
                )
            if ci == 0:
                nc.vector.tensor_add(pt[:, :256], pt[:, :256], mb_sb[:])
            nc.scalar.activation(
                wt_sb[:, WOFF[g] + (q0 - 256 * g): WOFF[g] + (q0 - 256 * g) + w],
                pt[:, :w],
                mybir.ActivationFunctionType.Exp,
                scale=float(SCALE),
                accum_out=zp[:, ci:ci + 1],
            )
        vptile = vppool.tile([128, D], BF, tag="vp", name="vp")
        z = zpool.tile([128, 1], F32, tag="z", name="z")
        nc.vector.tensor_reduce(z[:], zp[:], axis=mybir.AxisListType.X,
                                op=mybir.AluOpType.add)
        rz = zpool.tile([128, 1], F32, tag="rz", name="rz")
        nc.vector.reciprocal(rz[:], z[:])
        nc.vector.tensor_scalar_mul(vptile[:], vt[g][:], rz[:])
        vp[g] = vptile

        if g >= 1:
            emit_av(g - 1)
    emit_av(G - 1)


def _emit_body_v3(nc, tc, ctx, pools, aps, lag=1):
    """v3: same dataflow as v1, but every PSUM accumulation chain is
    interleaved with >=2 other chains across a single 8-bank PSUM pool
    (consecutive same-bank accumulating matmuls cost ~600ns on HW; spacing
    them >=3 instructions apart hides the bubble). Weights (a2/wvT) are
    persistent; AV for group g is optionally emitted after scores of group
    g+1 (lag=1) so the exp/Z/vp chain hides under score matmuls."""
    (xpool, ypool, vpool, vppool, zpool, stpool, ps) = pools
    (xT_d, xkT_d, a2t, wvt, mb_sb, wt_sb, out_d) = aps

    xk = []
    for t in range(ET):
        xktile = xpool.tile([128, D], BF, tag="xk", name="xk")
        nc.sync.dma_start(xktile[:], xkT_d[t * 128:(t + 1) * 128, :])
        xk.append(xktile)
    xt = []
    for t in range(ET):
        xtile = xpool.tile([128, N], BF, tag="xt", name="xt")
        nc.sync.dma_start(xtile[:], xT_d[t * 128:(t + 1) * 128, :])
        xt.append(xtile)

    # ---- y projection: 16 chunks in 4 interleaved batches of 4 ----
    yt = [ypool.tile([128, D], BF, tag="yt", name="yt") for _ in range(ET)]
    vt = [vpool.tile([128, D], F32, tag="v", name="vt") for _ in range(G)]
    yv_chunks = [("y", dt_, c) for dt_ in range(ET) for c in range(2)] + \
                [("v", g, c) for g in range(G) for c in range(2)]
    for b0 in range(0, len(yv_chunks), 4):
        batch = yv_chunks[b0:b0 + 4]
        pts = [ps.tile([128, 512], F32, tag="ps", name="pt") for _ in batch]
        for e in range(ET):
            for j, (kind, i, c) in enumerate(batch):
                if kind == "y":
                    lhs = a2t[e][:, i * 128:(i + 1) * 128]
                    rhs = xk[e][:, c * 512:(c + 1) * 512]
                else:
                    lhs = xk[e][:, i * 128:(i + 1) * 128]
                    rhs = wvt[e][:, c * 512:(c + 1) * 512]
                nc.tensor.matmul(pts[j][:], lhs, rhs,
                                 start=(e == 0), stop=(e == ET - 1))
        for j, (kind, i, c) in enumerate(batch):
            dst = yt[i] if kind == "y" else vt[i]
            nc.vector.tensor_copy(dst[:, c * 512:(c + 1) * 512], pts[j][:])

    # ---- scores: group-major, chunks of a set interleaved ----
    SETS = [[0], [1], [2], [3], [4, 5, 6, 7]]
    vp = [None] * G
    zp = [None] * G

    def emit_av(g):
        # 4 interleaved chains: (qt, oc) pairs
        qts = (2 * g, 2 * g + 1)
        stages = {qt: stpool.tile([128, D], F32, tag="st", name="st")
                  for qt in qts}
        pts_av = {(qt, oc): ps.tile([128, 512], F32, tag="ps", name="apt")
                  for qt in qts for oc in range(2)}
        for gg in range(g + 1):
            for qt in qts:
                lhs = wt_sb[:, WOFF[gg] + 128 * qt - 256 * gg:
                               WOFF[gg] + 128 * qt - 256 * gg + 128]
                for oc in range(2):
                    nc.tensor.matmul(pts_av[(qt, oc)][:], lhs,
                                     vp[gg][:, oc * 512:(oc + 1) * 512],
                                     start=(gg == 0), stop=(gg == g))
        for qt in qts:
            for oc in range(2):
                nc.vector.tensor_copy(stages[qt][:, oc * 512:(oc + 1) * 512],
                                      pts_av[(qt, oc)][:])
            nc.sync.dma_start(out_d[qt * 128:(qt + 1) * 128, :], stages[qt][:])

    prev_groups = []
    for si, gset in enumerate(SETS):
        # chains: (g, ci, q0, w, psum)
        chains = []
        for g in gset:
            ch = _score_chunks(g)
            zp[g] = zpool.tile([128, len(ch)], F32, tag="zp", name="zp")
            for ci, (q0, w) in enumerate(ch):
                chains.append((g, ci, q0, w,
                               ps.tile([128, 512], F32, tag="ps", name="spt")))
        for dt_ in range(ET):
            for (g, ci, q0, w, pt) in chains:
                nc.tensor.matmul(
                    pt[:, :w],
                    yt[dt_][:, g * 128:(g + 1) * 128],
                    xt[dt_][:, q0:q0 + w],
                    start=(dt_ == 0), stop=(dt_ == ET - 1),
                )
        for (g, ci, q0, w, pt) in chains:
            if ci == 0:
                nc.vector.tensor_add(pt[:, :256], pt[:, :256], mb_sb[:])
            nc.scalar.activation(
                wt_sb[:, WOFF[g] + (q0 - 256 * g): WOFF[g] + (q0 - 256 * g) + w],
                pt[:, :w],
                mybir.ActivationFunctionType.Exp,
                scale=float(SCALE),
                accum_out=zp[g][:, ci:ci + 1],
            )
        for g in gset:
            vptile = vppool.tile([128, D], BF, tag="vp", name="vp")
            z = zpool.tile([128, 1], F32, tag="z", name="z")
            nc.vector.tensor_reduce(z[:], zp[g][:], axis=mybir.AxisListType.X,
                                    op=mybir.AluOpType.add)
            rz = zpool.tile([128, 1], F32, tag="rz", name="rz")
            nc.vector.reciprocal(rz[:], z[:])
            nc.vector.tensor_scalar_mul(vptile[:], vt[g][:], rz[:])
            vp[g] = vptile
        if lag:
            for g in prev_groups:
                emit_av(g)
            prev_groups = gset
        else:
            for g in gset:
                emit_av(g)
    for g in prev_groups:
        emit_av(g)


def build_program_v3(with_loop=False, max_iters=64, lag=1):
    nc = bacc.Bacc("TRN2", target_bir_lowering=False, debug=False, num_devices=8)
    xT_d = nc.dram_tensor("xT", [D, N], BF, kind="ExternalInput").ap()
    xkT_d = nc.dram_tensor("xkT", [D, D], BF, kind="ExternalInput").ap()
    a2_d = nc.dram_tensor("a2", [D, D], BF, kind="ExternalInput").ap()
    wvT_d = nc.dram_tensor("wvT", [D, D], BF, kind="ExternalInput").ap()
    mb_d = nc.dram_tensor("maskbias", [128, 256], F32, kind="ExternalInput").ap()
    out_d = nc.dram_tensor("out", [N, D], F32, kind="ExternalOutput").ap()
    if with_loop:
        n_d = nc.dram_tensor("niter", [1, 1], mybir.dt.int32,
                             kind="ExternalInput").ap()

    with tile.TileContext(nc) as tc:
        with ExitStack() as ctx:
            persist = ctx.enter_context(tc.tile_pool(name="persist", bufs=1))
            xpool = ctx.enter_context(tc.tile_pool(name="xT", bufs=ET))
            ypool = ctx.enter_context(tc.tile_pool(name="yT", bufs=ET))
            vpool = ctx.enter_context(tc.tile_pool(name="v", bufs=G))
            vppool = ctx.enter_context(tc.tile_pool(name="vp", bufs=G))
            zpool = ctx.enter_context(tc.tile_pool(name="z", bufs=3 * G))
            stpool = ctx.enter_context(tc.tile_pool(name="stage", bufs=4))
            ps = ctx.enter_context(tc.tile_pool(name="ps", bufs=8, space="PSUM"))

            mb_sb = persist.tile([128, 256], F32, tag="mb")
            nc.sync.dma_start(mb_sb[:], mb_d[:])
            wt_sb = persist.tile([128, WTOT], BF, tag="wt")
            a2t, wvt = [], []
            for t in range(ET):
                w1 = persist.tile([128, D], BF, tag=f"a2_{t}", name=f"a2_{t}")
                nc.sync.dma_start(w1[:], a2_d[t * 128:(t + 1) * 128, :])
                a2t.append(w1)
            for t in range(ET):
                w2 = persist.tile([128, D], BF, tag=f"wv_{t}", name=f"wv_{t}")
                nc.sync.dma_start(w2[:], wvT_d[t * 128:(t + 1) * 128, :])
                wvt.append(w2)

            pools = (xpool, ypool, vpool, vppool, zpool, stpool, ps)
            aps = (xT_d, xkT_d, a2t, wvt, mb_sb, wt_sb, out_d)

            if with_loop:
                n_sb = persist.tile([1, 1], mybir.dt.int32, tag="niter")
                nc.sync.dma_start(n_sb[:], n_d[:])
                regs = []
                with tc.tile_critical():
                    for e, eng in nc.engines.items():
                        r = eng.alloc_register(f"niter_{e.name}")
                        eng.reg_load(r, n_sb[0:1, 0:1])
                        regs.append(r)
                n_val = nc.snap(bass.RegisterHandles(regs), min_val=0,
                                max_val=max_iters)
                with tc.For_i(0, n_val, 1):
                    _emit_body_v3(nc, tc, ctx, pools, aps, lag)
            else:
                _emit_body_v3(nc, tc, ctx, pools, aps, lag)

    nc.compile()
    return nc


def build_program_v2(with_loop=False, max_iters=64):
    nc = bacc.Bacc("TRN2", target_bir_lowering=False, debug=False, num_devices=8)
    xT_d = nc.dram_tensor("xT", [D, N], BF, kind="ExternalInput").ap()
    xkT_d = nc.dram_tensor("xkT", [D, D], BF, kind="ExternalInput").ap()
    a2_d = nc.dram_tensor("a2", [D, D], BF, kind="ExternalInput").ap()
    wvT_d = nc.dram_tensor("wvT", [D, D], BF, kind="ExternalInput").ap()
    mb_d = nc.dram_tensor("maskbias", [128, 256], F32, kind="ExternalInput").ap()
    out_d = nc.dram_tensor("out", [N, D], F32, kind="ExternalOutput").ap()
    if with_loop:
        n_d = nc.dram_tensor("niter", [1, 1], mybir.dt.int32,
                             kind="ExternalInput").ap()

    with tile.TileContext(nc) as tc:
        with ExitStack() as ctx:
            persist = ctx.enter_context(tc.tile_pool(name="persist", bufs=1))
            xpool = ctx.enter_context(tc.tile_pool(name="xT", bufs=ET))
            ypool = ctx.enter_context(tc.tile_pool(name="yT", bufs=ET))
            vpool = ctx.enter_context(tc.tile_pool(name="v", bufs=G))
            vppool = ctx.enter_context(tc.tile_pool(name="vp", bufs=G))
            zpool = ctx.enter_context(tc.tile_pool(name="z", bufs=3 * G))
            stpool = ctx.enter_context(tc.tile_pool(name="stage", bufs=4))
            ps = ctx.enter_context(tc.tile_pool(name="ps", bufs=4, space="PSUM"))
            avps = ctx.enter_context(tc.tile_pool(name="avps", bufs=4, space="PSUM"))

            mb_sb = persist.tile([128, 256], F32, tag="mb")
            nc.sync.dma_start(mb_sb[:], mb_d[:])
            wt_sb = persist.tile([128, WTOT], BF, tag="wt")
            # persistent weights: loaded once, reused every iteration
            a2t, wvt = [], []
            for t in range(ET):
                w1 = persist.tile([128, D], BF, tag=f"a2_{t}", name=f"a2_{t}")
                nc.sync.dma_start(w1[:], a2_d[t * 128:(t + 1) * 128, :])
                a2t.append(w1)
            for t in range(ET):
                w2 = persist.tile([128, D], BF, tag=f"wv_{t}", name=f"wv_{t}")
                nc.sync.dma_start(w2[:], wvT_d[t * 128:(t + 1) * 128, :])
                wvt.append(w2)

            pools = (xpool, ypool, vpool, vppool, zpool, stpool, ps, avps)
            aps = (xT_d, xkT_d, a2t, wvt, mb_sb, wt_sb, out_d)

            if with_loop:
                n_sb = persist.tile([1, 1], mybir.dt.int32, tag="niter")
                nc.sync.dma_start(n_sb[:], n_d[:])
                regs = []
                with tc.tile_critical():
                    for e, eng in nc.engines.items():
                        r = eng.alloc_register(f"niter_{e.name}")
                        eng.reg_load(r, n_sb[0:1, 0:1])
                        regs.append(r)
                n_val = nc.snap(bass.RegisterHandles(regs), min_val=0,
                                max_val=max_iters)
                with tc.For_i(0, n_val, 1):
                    _emit_body_v2(nc, tc, ctx, pools, aps)
            else:
                _emit_body_v2(nc, tc, ctx, pools, aps)

    nc.compile()
    return nc


def build_program(with_loop=False, max_iters=64, stages="full"):
    """Build and compile the SPMD program. Returns the compiled Bacc."""
    nc = bacc.Bacc("TRN2", target_bir_lowering=False, debug=False, num_devices=8)
    xT_d = nc.dram_tensor("xT", [D, N], BF, kind="ExternalInput").ap()
    xkT_d = nc.dram_tensor("xkT", [D, D], BF, kind="ExternalInput").ap()
    a2_d = nc.dram_tensor("a2", [D, D], BF, kind="ExternalInput").ap()
    wvT_d = nc.dram_tensor("wvT", [D, D], BF, kind="ExternalInput").ap()
    mb_d = nc.dram_tensor("maskbias", [128, 256], F32, kind="ExternalInput").ap()
    out_d = nc.dram_tensor("out", [N, D], F32, kind="ExternalOutput").ap()
    if with_loop:
        n_d = nc.dram_tensor("niter", [1, 1], mybir.dt.int32,
                             kind="ExternalInput").ap()

    with tile.TileContext(nc) as tc:
        with ExitStack() as ctx:
            persist = ctx.enter_context(tc.tile_pool(name="persist", bufs=1))
            xpool = ctx.enter_context(tc.tile_pool(name="xT", bufs=ET))
            wpool = ctx.enter_context(tc.tile_pool(name="weights", bufs=2 * ET))
            ypool = ctx.enter_context(tc.tile_pool(name="yT", bufs=ET))
            vpool = ctx.enter_context(tc.tile_pool(name="v", bufs=G))
            vppool = ctx.enter_context(tc.tile_pool(name="vp", bufs=G))
            zpool = ctx.enter_context(tc.tile_pool(name="z", bufs=3 * G))
            stpool = ctx.enter_context(tc.tile_pool(name="stage", bufs=4))
            ps = ctx.enter_context(tc.tile_pool(name="ps", bufs=4, space="PSUM"))
            avps = ctx.enter_context(tc.tile_pool(name="avps", bufs=4, space="PSUM"))

            mb_sb = persist.tile([128, 256], F32, tag="mb")
            nc.sync.dma_start(mb_sb[:], mb_d[:])
            wt_sb = persist.tile([128, WTOT], BF, tag="wt")
            mm_tiles = None
            if stages != "full":
                nc.vector.memset(wt_sb[:], 0.0)
                ymm = [persist.tile([128, D], BF, tag=f"ymm{i}",
                                    name=f"ymm{i}") for i in range(ET)]
                vmm = [persist.tile([128, D], BF, tag=f"vmm{i}",
                                    name=f"vmm{i}") for i in range(G)]
                for tl_ in ymm + vmm:
                    nc.vector.memset(tl_[:], 0.0)
                mm_tiles = (ymm, vmm)

            pools = (xpool, wpool, ypool, vpool, vppool, zpool, stpool, ps, avps)
            aps = (xT_d, xkT_d, a2_d, wvT_d, mb_sb, wt_sb, out_d)

            if with_loop:
                n_sb = persist.tile([1, 1], mybir.dt.int32, tag="niter")
                nc.sync.dma_start(n_sb[:], n_d[:])
                regs = []
                with tc.tile_critical():
                    for e, eng in nc.engines.items():
                        r = eng.alloc_register(f"niter_{e.name}")
                        eng.reg_load(r, n_sb[0:1, 0:1])
                        regs.append(r)
                n_val = nc.snap(bass.RegisterHandles(regs), min_val=0,
                                max_val=max_iters)
                with tc.For_i(0, n_val, 1):
                    _emit_body(nc, tc, ctx, pools, aps, stages, mm_tiles)
            else:
                _emit_body(nc, tc, ctx, pools, aps, stages, mm_tiles)

    nc.compile()
    return nc


def prepare_in_maps(x, Wq, Wk, Wv, niter=None):
    """Host-side sharding: per-core input maps (8 cores)."""
    x = np.asarray(x, dtype=np.float32)
    A2 = (np.asarray(Wk, np.float32).T @ np.asarray(Wq, np.float32))  # [e, d]
    a2_bf = A2.astype(ml_dtypes.bfloat16)
    wvT_bf = np.asarray(Wv, np.float32).T.astype(ml_dtypes.bfloat16)  # [e, o]
    mb = []
    for h in range(2):
        i = np.arange(128)[:, None]
        j = np.arange(256)[None, :]
        mb.append(np.where(j >= 2 * i + h, 0.0, NEGBIG).astype(np.float32))
    in_maps = []
    for c in range(8):
        b, h = c // 2, c % 2
        xTb = x[b].T.astype(ml_dtypes.bfloat16)
        m = {
            "xT": xTb,
            "xkT": np.ascontiguousarray(xTb[:, h::2]),
            "a2": a2_bf,
            "wvT": wvT_bf,
            "maskbias": mb[h],
        }
        if niter is not None:
            m["niter"] = np.array([[niter]], dtype=np.int32)
        in_maps.append(m)
    return in_maps


_CACHE = {}


def kernel(x, Wq, Wk, Wv):
    if "nc" not in _CACHE:
        _CACHE["nc"] = build_program(with_loop=False)
    nc = _CACHE["nc"]
    in_maps = prepare_in_maps(x, Wq, Wk, Wv)
    res = run_bass_kernel_spmd(nc, in_maps, list(range(8)), trace=False)
    out = np.empty((B, N, D), np.float32)
    for b in range(B):
        out[b] = res.results[2 * b]["out"] + res.results[2 * b + 1]["out"]
    return out

